# revision 3
# baseline (speedup 1.0000x reference)
"""Trainium2 Bass kernel for DatasetIndexedTopK (streaming top-k retrieval).

Problem: scores = Q @ C^T with Q [512, 128], C [1M, 128]; return per-query
top-100 (scores, ids), matching jax.lax.top_k semantics (ties -> lower id).

Design (8-way shard over candidates, 131072 per core):
  Device (per core): stream candT in 8192-wide tiles; f32r matmuls (full PE
  rate, max |score err| ~9e-3 measured) fill [128, 2048] PSUM tiles; the DVE
  reduces each PSUM tile directly (tensor_reduce max over innermost 32) into
  a bf16 cell-max summary S [128, 4096] per 128-query chunk.  Only the
  per-32-candidate maxima leave the device: out_cm [512, 4096] bf16.

  Host: concat the 8 cores' summaries -> [512, 32768] cell maxima.  The
  100th-largest cell-max v100 >= s_100 - 0.26 (bf16 rounding 0.125 + f32r
  error 0.01, both ways), so every cell containing a true top-100 candidate
  has summary >= v100 - 0.52.  Select cells >= v100 - 0.7 (provably a
  superset), gather their 32 candidates each, rescore exactly in fp32, and
  take the exact top-k with the reference's tie order (score desc, id asc).
"""

import numpy as np

P = 128                 # SBUF partitions / queries per chunk
D = 128                 # embedding dim (contraction)
Q = 512                 # queries
NCORES = 8
NCAND_TOTAL = 256 * 4096
NCAND = NCAND_TOTAL // NCORES    # 131072 candidates per core
CTILE = 8192            # candidate columns per DMA tile
PBLK = 2048             # columns per PSUM tile (4 banks)
CELL = 32               # candidates per summary cell
NCELL = NCAND // CELL   # 4096 cells per core
NCHUNK = Q // P         # 4 query chunks
MARGIN = 0.7            # cell-selection slack (> 2*(bf16 ulp + f32r err))

_CACHE = {}


def _build_bass():
    import concourse.bacc as bacc
    import concourse.mybir as mybir
    from concourse.tile import TileContext
    from contextlib import ExitStack

    f32 = mybir.dt.float32
    f32r = mybir.dt.float32r
    bf16 = mybir.dt.bfloat16
    ncell_span = PBLK // CELL          # 64
    nspan = CTILE // PBLK              # 4

    nc = bacc.Bacc()
    qT = nc.declare_dram_parameter("qT", [D, Q], f32r, isOutput=False)
    candT = nc.declare_dram_parameter("candT", [D, NCAND], f32r, isOutput=False)
    out_cm = nc.declare_dram_parameter("out_cm", [Q, NCELL], bf16, isOutput=True)

    with ExitStack() as ctx:
        tc = ctx.enter_context(TileContext(nc))
        qpool = ctx.enter_context(tc.tile_pool(name="q", bufs=1))
        cpool = ctx.enter_context(tc.tile_pool(name="cand", bufs=3))
        pspool = ctx.enter_context(tc.tile_pool(name="ps", bufs=2, space="PSUM"))
        acc = ctx.enter_context(tc.tile_pool(name="acc", bufs=1))

        qsb = qpool.tile([D, Q], f32r, tag="qsb")
        nc.sync.dma_start(qsb[:], qT[:])

        S_all = acc.tile([P, NCHUNK * NCELL], bf16, tag="S")

        for t in range(NCAND // CTILE):
            ct = cpool.tile([D, CTILE], f32r, tag="cand")
            nc.sync.dma_start(ct[:], candT[:, t * CTILE:(t + 1) * CTILE])
            for qc in range(NCHUNK):
                for sp in range(nspan):
                    ps = pspool.tile([P, ncell_span, CELL], f32, tag="ps")
                    for j in range(PBLK // 512):
                        col = sp * PBLK + j * 512
                        npc = 512 // CELL
                        nc.tensor.matmul(
                            ps[:, j * npc:(j + 1) * npc, :],
                            lhsT=qsb[:, qc * P:(qc + 1) * P],
                            rhs=ct[:, col: col + 512],
                            start=True,
                            stop=True,
                        )
                    so = qc * NCELL + t * (CTILE // CELL) + sp * ncell_span
                    nc.vector.tensor_reduce(
                        out=S_all[:, so:so + ncell_span], in_=ps[:],
                        axis=mybir.AxisListType.X, op=mybir.AluOpType.max,
                    )

        for qc in range(NCHUNK):
            nc.sync.dma_start(
                out_cm[qc * P:(qc + 1) * P, :],
                S_all[:, qc * NCELL:(qc + 1) * NCELL],
            )
    nc.compile()
    return nc


def _get_bass():
    if "nc" not in _CACHE:
        _CACHE["nc"] = _build_bass()
    return _CACHE["nc"]


def kernel(query_embeddings, candidate_embeddings, candidate_indices, k):
    from concourse.bass_utils import run_bass_kernel_spmd

    q = np.ascontiguousarray(np.asarray(query_embeddings, dtype=np.float32))
    c = np.asarray(candidate_embeddings, dtype=np.float32).reshape(NCAND_TOTAL, D)
    ids_flat = np.asarray(candidate_indices).reshape(-1)
    k = int(k)
    assert k <= 1024

    qT = np.ascontiguousarray(q.T)                       # [128, 512]
    in_maps = []
    for core in range(NCORES):
        shard = c[core * NCAND:(core + 1) * NCAND]       # [131072, 128]
        in_maps.append({
            "qT": qT,
            "candT": np.ascontiguousarray(shard.T),      # [128, 131072]
        })

    nc = _get_bass()
    res = run_bass_kernel_spmd(nc, in_maps, core_ids=list(range(NCORES))).results

    # ---- host: exact top-k from cell-max summaries ----
    cm = np.concatenate(
        [res[core]["out_cm"].astype(np.float32) for core in range(NCORES)],
        axis=1,
    )                                                    # [512, 32768]
    vk = np.partition(cm, -k, axis=1)[:, -k]             # kth-largest cell max
    tau = vk - MARGIN
    counts = (cm >= tau[:, None]).sum(axis=1)
    K = int(counts.max())
    sel_cells = np.argpartition(-cm, K - 1, axis=1)[:, :K]   # [512, K]

    # global candidate positions of each selected cell's 32 members
    core_of = sel_cells >> 12                            # // 4096
    local = sel_cells & 0xFFF
    base = core_of * NCAND + local * CELL                # [512, K]
    pos = (base[:, :, None] + np.arange(CELL)[None, None, :]).reshape(Q, K * CELL)

    out_scores = np.empty((Q, k), dtype=np.float32)
    out_pos = np.empty((Q, k), dtype=np.int64)
    QB = 64                                              # query batch (memory cap)
    for q0 in range(0, Q, QB):
        q1 = min(q0 + QB, Q)
        sel = c[pos[q0:q1]]                              # [qb, K*32, 128]
        sc = np.einsum("qnd,qd->qn", sel, q[q0:q1], optimize=True)
        for qi in range(q0, q1):
            row = sc[qi - q0]
            p = pos[qi]
            # exact order among a slightly larger head to honor tie-break
            head = np.argpartition(-row, min(k + 32, row.size - 1))[:k + 32]
            order = head[np.lexsort((p[head], -row[head]))][:k]
            out_scores[qi] = row[order]
            out_pos[qi] = p[order]

    out_ids = ids_flat[out_pos].astype(ids_flat.dtype)
    return out_scores, out_ids
